# revision 72
# baseline (speedup 1.0000x reference)
"""Trainium2 Bass kernel: 16-head attention block (B=2, S=2048, H=1024).

Sharding: 8 cores = 2-way data parallel (batch) x 4-way tensor parallel
(head groups of 4 heads / 256 dims). Each core computes, for its batch
and head group:
    Q^T, K^T (= W @ x^T, [dims, seq] layout; Wq/bq pre-scaled by 1/8 on
    host so no score scaling is needed on device), V ([seq, dims]),
    S^T = K Q^T per head (key positions on partitions),
    P^T = exp(S^T + mask),
    ctx'^T = [V | 1]^T P^T    (ones column folded in -> row 64 = softmax
                               denominator),
    ctx^T normalized, then partial output O_g = ctx^T.T @ Wo[:,hs]^T.
Host sums the 4 partial outputs per batch and adds bo.

x / Wk / Wq / Wv and the partial output travel as bf16 (same PE matmul
rate as float32r, half the DMA bytes; all accumulation stays fp32 in
PSUM).  K/Q/V/ctx^T activations and Wo stay float32r.

Scheduling: one software-pipelined stream.  The softmax exp stream on
the scalar engine (ACT) and the PE are co-saturated mid-kernel, so the
emission keeps both fed: score matmuls flow continuously across chunk
boundaries while ctx matmuls lag EIGHT key-tiles behind (a deep
elastic buffer of exp'd tiles in the pt pool, so the PE never stalls
on an exp); each chunk's normalization is emitted as filler inside the
next chunk, with the reciprocal broadcast matmuls placed in proj-tag
PSUM slots so the score/ctx tags stay strictly chunk-ordered (the ctx
PSUM tiles are created lazily at the first ctx matmul of each chunk to
keep the bufs=1 tag rotation legal).  Projections for the second head
pair (those not already hoisted into phase A's DMA-stall windows) and
the output projection of the previous chunk ride in ns-budgeted filler
slots inside the kt loops.  The final chunk's tail is specialized:
its numerator copies run on the then-idle scalar engine (overlapping
the DVE reciprocals), its head-o contribution is contracted straight
out of the staging tile against a second copy of those Wo rows loaded
at partitions 0:64 (wo2) -- skipping the cross-partition staging DMA
-- and its output DMAs are SP-issued whole m-tiles with scalar-engine
PSUM->SBUF copies for the second halves.
"""

import contextlib
from collections import deque

import numpy as np

import concourse.bass as bass
import concourse.mybir as mybir
import concourse.tile as tile
from concourse import bacc
from concourse.bass_utils import run_bass_kernel_spmd

B, S, H = 2, 2048, 1024
NUM_HEADS, HEAD_DIM = 16, 64
N_CORES = 8
GROUPS = 4                  # head-parallel groups per batch
HD = H // GROUPS            # 256 head-dims per core (4 heads)
P = 128
KT_H = H // P               # 8 k-tiles over hidden dim
KT_S = S // P               # 16 k-tiles over sequence (key positions)
NCH = 4                     # q chunks
CHUNK = S // NCH            # 512
F32 = mybir.dt.float32
F32R = mybir.dt.float32r
BF16 = mybir.dt.bfloat16
EXP = mybir.ActivationFunctionType.Exp

_PROGRAM_CACHE = {}


class _Emitter:
    def __init__(self, tc, nc, dram, masked, with_bias):
        self.tc, self.nc = tc, nc
        self.masked, self.with_bias = masked, with_bias
        (self.xT_d, self.wq_d, self.wk_d, self.wv_d, self.wo_d,
         self.bq_d, self.bk_d, self.bv_d, self.am_d, self.o_d) = dram
        self.filler = deque()   # (cost_ns, thunk) pending filler ops
        self.debt = 0.0         # accumulated filler budget (ns of PE work)
        self.pend = deque()     # (p, kt, pt, ctx_e, ctx_o, after)

    # ---------------- filler queue ----------------
    def filler_step(self, budget_ns=0.0):
        self.debt = min(self.debt + budget_ns, 2000.0)
        while self.filler:
            cost, thunk = self.filler[0]
            if cost == 0 or cost <= self.debt:
                self.filler.popleft()
                thunk()
                self.debt -= cost
            else:
                break
        if self.debt < 0:
            self.debt = 0.0

    def drain_filler(self):
        while self.filler:
            self.filler.popleft()[1]()

    # ---------------- projection building blocks ----------------
    def qk_cc_mms(self, ps, w_sb, pair, cc, kts):
        nc = self.nc
        for kt in kts:
            nc.tensor.matmul(
                ps[:],
                w_sb[:, kt, pair * P:(pair + 1) * P],
                self.xT_sb[:, kt, cc * CHUNK:(cc + 1) * CHUNK],
                start=(kt == 0),
                stop=(not self.with_bias and kt == KT_H - 1))

    def qk_cc_finish(self, ps, b_sb, dst, pair, cc):
        nc = self.nc
        if self.with_bias:
            nc.tensor.matmul(ps[:], b_sb[:, pair * P:(pair + 1) * P],
                             self.ones_sb[:, 0:CHUNK], start=False, stop=True)
        nc.vector.tensor_copy(dst[:, pair, cc * CHUNK:(cc + 1) * CHUNK], ps[:])

    def qk_cc_proj(self, w_sb, b_sb, dst, pair, cc, tag):
        ps = self.psA.tile([P, CHUNK], F32, bufs=1, name="ps_" + tag, tag=tag)
        self.qk_cc_mms(ps, w_sb, pair, cc, range(KT_H))
        self.qk_cc_finish(ps, b_sb, dst, pair, cc)

    def v_one(self, m, tag):
        nc = self.nc
        ps = self.psA.tile([P, HD], F32, bufs=1, name="psv_" + tag, tag=tag)
        for kt in range(KT_H):
            nc.tensor.matmul(
                ps[:],
                self.xT_sb[:, kt, m * P:(m + 1) * P],
                self.wv_sb[:, kt, :],
                start=(kt == 0), stop=(not self.with_bias and kt == KT_H - 1))
        if self.with_bias:
            nc.tensor.matmul(ps[:], self.ones_sb[:, 0:P], self.bv_sb[:],
                             start=False, stop=True)
        nc.vector.tensor_copy(self.v_sb[:, m, :, 0:HEAD_DIM], ps[:])

    def queue_kq_pair1(self, cols):
        """Pair-1 K/Q projection column-chunks as fine-grained filler
        thunks (4 thunks of 2 accumulating matmuls each)."""
        for which, cc in cols:
            w_sb, b_sb, dst, tag = (
                (self.wk_sb, self.bk_sb, self.kT_sb, "ps_k") if which == "k"
                else (self.wq_sb, self.bq_sb, self.qT_sb, "ps_q"))
            state = {}
            def t0(state=state, w_sb=w_sb, tag=tag, cc=cc):
                state["ps"] = self.psA.tile([P, CHUNK], F32, bufs=1,
                                            name="ps_" + tag, tag=tag)
                self.qk_cc_mms(state["ps"], w_sb, 1, cc, range(0, 2))
            def tmid(kts, state=state, w_sb=w_sb, cc=cc):
                self.qk_cc_mms(state["ps"], w_sb, 1, cc, kts)
            def tend(state=state, w_sb=w_sb, b_sb=b_sb, dst=dst, cc=cc):
                self.qk_cc_mms(state["ps"], w_sb, 1, cc, range(6, KT_H))
                self.qk_cc_finish(state["ps"], b_sb, dst, 1, cc)
            self.filler.append((427, t0))
            self.filler.append((427, lambda kts=range(2, 4), f=tmid: f(kts)))
            self.filler.append((427, lambda kts=range(4, 6), f=tmid: f(kts)))
            self.filler.append((427, tend))

    def queue_oproj_quarter(self, q, act_copies=False, final=False):
        for m in range(4 * q, 4 * q + 4):
            self.queue_oproj_m(m, act_copies, final)

    def queue_oproj_m(self, m, act_copies=False, final=False):
        """Output projection for one seq m-tile as filler thunks.
        PSUM reuses the proj-pool tags; each half is copied out and
        DMA'd to DRAM immediately.  For the final quarter (``act_copies``)
        half the PSUM->SBUF copies run on the then-idle scalar engine so
        the tail is matmul-bound instead of DVE-copy-bound, and
        (``final``) the pair-1 head-o contribution is contracted straight
        out of the tmp_o staging tile against the wo2 copy of its Wo rows
        (both at partitions 0:64), skipping the cross-partition staging
        DMA entirely."""
        nc = self.nc
        if True:
            o_sb = self.opool.tile([P, H], BF16, tag="o_sb", bufs=3)
            for n2 in range(2):
                def mk_mm(m=m, n2=n2, o_sb=o_sb):
                    tag = "ps_k" if n2 == 0 else "ps_q"
                    ps_o = self.psA.tile([P, CHUNK], F32, tag=tag, bufs=1,
                                         name="pso_" + tag)
                    ncols = slice(n2 * CHUNK, (n2 + 1) * CHUNK)
                    if final:
                        nc.tensor.matmul(
                            ps_o[:],
                            self.ctxT_sb[:, 0, m * P:(m + 1) * P],
                            self.wo_sb[:, 0, ncols],
                            start=True, stop=False)
                        nc.tensor.matmul(
                            ps_o[:],
                            self.ctxT_sb[0:64, 1, m * P:(m + 1) * P],
                            self.wo_sb[0:64, 1, ncols],
                            start=False, stop=False)
                        mcols = slice((m - 4 * (NCH - 1)) * P,
                                      (m - 4 * (NCH - 1) + 1) * P)
                        nc.tensor.matmul(
                            ps_o[:],
                            self.tmp_o_final[0:64, mcols],
                            self.wo2_sb[:, ncols],
                            start=False, stop=True)
                    else:
                        for kt2 in range(HD // P):
                            nc.tensor.matmul(
                                ps_o[:],
                                self.ctxT_sb[:, kt2, m * P:(m + 1) * P],
                                self.wo_sb[:, kt2, n2 * CHUNK:(n2 + 1) * CHUNK],
                                start=(kt2 == 0), stop=(kt2 == HD // P - 1))
                    if act_copies and n2 == 1:
                        nc.scalar.copy(
                            o_sb[:, n2 * CHUNK:(n2 + 1) * CHUNK], ps_o[:])
                    else:
                        nc.vector.tensor_copy(
                            o_sb[:, n2 * CHUNK:(n2 + 1) * CHUNK], ps_o[:])
                    nc.sync.dma_start(
                        out=self.o_d[m * P:(m + 1) * P,
                                     n2 * CHUNK:(n2 + 1) * CHUNK],
                        in_=o_sb[:, n2 * CHUNK:(n2 + 1) * CHUNK])
                self.filler.append((427, mk_mm))

    # ---------------- attention ----------------
    def emit_pend_ctx(self):
        """Pop one pending kt: emit its two ctx matmuls.  The chunk's ctx
        PSUM tiles are created lazily HERE (first pop of the chunk), so
        with bufs=1 tags the previous chunk's ctx matmuls are always
        fully emitted before the next chunk's tiles rotate the bank."""
        if not self.pend:
            return
        p, kt, pt, state, after = self.pend.popleft()
        if state["ctx"] is None:
            state["ctx"] = (
                self.a_ps.tile([HEAD_DIM + 1, CHUNK], F32, tag="ctx_e",
                               bufs=1, name="ctx_e"),
                self.a_ps.tile([HEAD_DIM + 1, CHUNK], F32, tag="ctx_o",
                               bufs=1, name="ctx_o"))
        ctx_e, ctx_o = state["ctx"]
        mm = self.nc.tensor.matmul
        for hl in range(2):
            mm((ctx_e if hl == 0 else ctx_o)[:],
               self.v_sb[:, kt, 2 * p + hl, :],
               pt[:, hl * CHUNK:(hl + 1) * CHUNK],
               start=(kt == 0), stop=(kt == KT_S - 1))
        if after is not None:
            after(state)

    def attn_step(self, p, c, kt, state, budget=0.0, after=None):
        """Scores + exp for (p, c, kt); ctx lags two kt behind so the PE
        runs behind the ACT-bound softmax and never makes it wait."""
        nc = self.nc
        mm = nc.tensor.matmul
        s_pair = self.a_ps.tile([P, 2 * CHUNK], F32, tag="s_pair", bufs=2)
        for hl in range(2):
            mm(s_pair[:, hl * CHUNK:(hl + 1) * CHUNK],
               self.kT_sb[hl * 64:(hl + 1) * 64, p, kt * P:(kt + 1) * P],
               self.qT_sb[hl * 64:(hl + 1) * 64, p, c * CHUNK:(c + 1) * CHUNK],
               start=True, stop=True)
        pt = self.ptp.tile([P, 2 * CHUNK], F32R, tag="pt")
        if self.masked:
            nc.scalar.activation(pt[:], s_pair[:], EXP,
                                 bias=self.amask_sb[:, kt:kt + 1])
        else:
            nc.scalar.activation(pt[:], s_pair[:], EXP)
        self.pend.append((p, kt, pt, state, after))
        if len(self.pend) > 10:
            self.emit_pend_ctx()
        self.filler_step(budget)

    def make_norm_cb(self, p, c, oproj_q=None, split=False):
        """Callback run right after this chunk's last ctx matmul: emit
        the DVE part of the normalization (reciprocals straight from the
        PSUM denominator rows, then the numerator copies), and queue the
        PE broadcast + multiplies (+ the chunk's output projection, in
        phase C) as filler."""
        nc = self.nc

        def cb(state):
            ctx_e, ctx_o = state["ctx"]
            recip = self.npool.tile([HEAD_DIM + 1, 2, CHUNK], F32R,
                                    tag="recip", bufs=2)
            with nc.allow_low_precision(reason="softmax denominators are O(1e3); 11-bit mantissa is plenty"):
                nc.vector.reciprocal(recip[64:65, 0, :], ctx_e[64:65, :])
                nc.vector.reciprocal(recip[64:65, 1, :], ctx_o[64:65, :])
            ctxu = self.npool.tile([HEAD_DIM, 2, CHUNK], F32, tag="ctxu",
                                   bufs=2)
            nc.vector.tensor_copy(ctxu[:, 0, :], ctx_e[0:64, :])
            nc.vector.tensor_copy(ctxu[:, 1, :], ctx_o[0:64, :])

            def post():
                # reciprocal broadcasts across the 64 dim partitions; live
                # in proj-tag PSUM slots so the score/ctx tags stay purely
                # chunk-ordered
                bc_e = self.psA.tile([P, CHUNK], F32, tag="ps_k", bufs=1,
                                     name="bc_e")
                bc_o = self.psA.tile([P, CHUNK], F32, tag="ps_q", bufs=1,
                                     name="bc_o")
                for hl in range(2):
                    nc.tensor.matmul((bc_e if hl == 0 else bc_o)[0:HEAD_DIM, :],
                                     self.ones64[64:65, :],
                                     recip[64:65, hl, :],
                                     start=True, stop=True)
                nc.vector.tensor_mul(
                    self.ctxT_sb[0:64, p, c * CHUNK:(c + 1) * CHUNK],
                    ctxu[:, 0, :], bc_e[0:64, :])
                tmp_o = self.npool.tile([HEAD_DIM, CHUNK], F32R,
                                        tag="tmp_o", bufs=2)
                nc.vector.tensor_mul(tmp_o[:], ctxu[:, 1, :], bc_o[0:64, :])
                nc.sync.dma_start(
                    out=self.ctxT_sb[64:128, p, c * CHUNK:(c + 1) * CHUNK],
                    in_=tmp_o[:])

            self.filler.appendleft((427, post))
            if oproj_q is not None:
                self.queue_oproj_quarter(oproj_q,
                                         act_copies=(oproj_q == NCH - 1))

        def cb_final(state):
            """Final-chunk variant: full-width reciprocals, copies and
            broadcasts, but the normalization multiplies + head-o staging
            DMA are emitted in 128-column slices, with each output-
            projection m-tile queued to chase its own slice -- so the
            final projection starts as soon as its columns land instead
            of after the whole chunk's staging DMA."""
            ctx_e, ctx_o = state["ctx"]
            recip = self.npool.tile([HEAD_DIM + 1, 2, CHUNK], F32R,
                                    tag="recip", bufs=2)
            with nc.allow_low_precision(reason="softmax denominators are O(1e3); 11-bit mantissa is plenty"):
                nc.vector.reciprocal(recip[64:65, 0, :], ctx_e[64:65, :])
                nc.vector.reciprocal(recip[64:65, 1, :], ctx_o[64:65, :])

            # numerator copies ride the idle scalar engine, overlapping
            # the DVE reciprocals
            ctxu = self.npool.tile([HEAD_DIM, 2, CHUNK], F32, tag="ctxu",
                                   bufs=2)
            nc.scalar.copy(ctxu[:, 0, :], ctx_e[0:64, :])
            nc.scalar.copy(ctxu[:, 1, :], ctx_o[0:64, :])

            def post_final():
                # score-tag banks are free after the last exp; using them
                # keeps the proj tags clear for the output projection
                bc = self.a_ps.tile([P, 2 * CHUNK], F32, tag="s_pair",
                                    bufs=2, name="bc")
                for hl in range(2):
                    nc.tensor.matmul(
                        bc[0:HEAD_DIM, hl * CHUNK:(hl + 1) * CHUNK],
                        self.ones64[64:65, :],
                        recip[64:65, hl, :],
                        start=True, stop=True)
                bc_e = bc[:, 0:CHUNK]
                bc_o = bc[:, CHUNK:2 * CHUNK]
                base = c * CHUNK
                tmp_o = self.npool.tile([HEAD_DIM, CHUNK], F32R,
                                        tag="tmp_o", bufs=2)
                self.tmp_o_final = tmp_o
                for s in range(CHUNK // P):
                    lo, hi = s * P, (s + 1) * P
                    nc.vector.tensor_mul(
                        self.ctxT_sb[0:64, p, base + lo:base + hi],
                        ctxu[:, 0, lo:hi], bc_e[0:64, lo:hi])
                    nc.vector.tensor_mul(tmp_o[:, lo:hi], ctxu[:, 1, lo:hi],
                                         bc_o[0:64, lo:hi])

            self.filler.appendleft((427, post_final))
            self.queue_oproj_quarter(c, act_copies=True, final=True)

        return cb_final if split else cb

    def run_chunk(self, p, c, budget=640.0, oproj_q=None, split=False):
        state = {"ctx": None}
        cb = self.make_norm_cb(p, c, oproj_q, split=split)
        for kt in range(KT_S):
            self.attn_step(p, c, kt, state, budget,
                           after=(cb if kt == KT_S - 1 else None))

    # ---------------- main emission ----------------
    def emit(self):
        tc, nc = self.tc, self.nc
        stack = contextlib.ExitStack()
        with stack:
            const = stack.enter_context(tc.tile_pool(name="const", bufs=1))
            big = stack.enter_context(tc.tile_pool(name="big", bufs=1))

            onesf = const.tile([P, 64], F32)
            nc.any.memset(onesf[:], 1.0)
            ones64 = const.tile([P, 64], F32R)
            nc.vector.tensor_copy(ones64[:], onesf[:])
            self.ones64 = ones64
            # warm the ACT exp table before it is first needed
            trash = const.tile([1, 16], F32)
            nc.scalar.activation(trash[:], onesf[0:1, 0:16], EXP)
            if self.masked:
                self.amask_sb = const.tile([P, KT_S], F32)
                nc.sync.dma_start(out=self.amask_sb[:], in_=self.am_d[:])
            if self.with_bias:
                self.ones_sb = const.tile([1, CHUNK], BF16)
                for i in range(8):
                    nc.vector.tensor_copy(
                        self.ones_sb[0:1, i * 64:(i + 1) * 64], onesf[0:1, :])
                self.bq_sb = const.tile([1, HD], BF16)
                nc.sync.dma_start(out=self.bq_sb[:], in_=self.bq_d[:])
                self.bk_sb = const.tile([1, HD], BF16)
                nc.sync.dma_start(out=self.bk_sb[:], in_=self.bk_d[:])
                self.bv_sb = const.tile([1, HD], BF16)
                nc.sync.dma_start(out=self.bv_sb[:], in_=self.bv_d[:])
            else:
                self.bq_sb = self.bk_sb = self.bv_sb = None
                self.ones_sb = None

            # persistent activations
            self.qT_sb = big.tile([P, 2, S], F32R)
            self.kT_sb = big.tile([P, 2, S], F32R)
            self.v_sb = big.tile([P, KT_S, GROUPS, HEAD_DIM + 1], F32R)
            self.ctxT_sb = big.tile([P, 2, S], F32R)
            self.wo_sb = big.tile([P, HD // P, H], F32R)
            self.wo2_sb = big.tile([64, H], F32R)

            # ones column of V' (the rowsum trick)
            nc.vector.tensor_copy(self.v_sb[:, :, :, HEAD_DIM:HEAD_DIM + 1],
                                  onesf[:, 0:KT_S * GROUPS])

            # ---------- input tiles + DMAs ordered for earliest compute
            w_pool = tc.alloc_tile_pool(name="w_pool", bufs=1, side="right")
            self.wk_sb = w_pool.tile([P, KT_H, HD], BF16)
            self.wq_sb = w_pool.tile([P, KT_H, HD], BF16)
            self.xT_sb = w_pool.tile([P, KT_H, S], BF16)
            wv_stack = contextlib.ExitStack()
            wv_pool = wv_stack.enter_context(
                tc.tile_pool(name="wv_pool", bufs=1, side="right"))
            self.wv_sb = wv_pool.tile([P, KT_H, HD], BF16)

            wk_r = self.wk_d.rearrange("(t p) c -> p t c", p=P)
            wq_r = self.wq_d.rearrange("(t p) c -> p t c", p=P)
            wv_r = self.wv_d.rearrange("(t p) c -> p t c", p=P)
            xT_r = self.xT_d.rearrange("(t p) s -> p t s", p=P)

            nc.sync.dma_start(out=self.wk_sb[:, 0:1, :], in_=wk_r[:, 0:1, :])
            nc.sync.dma_start(
                out=self.xT_sb[:, 0, 0:CHUNK],
                in_=self.xT_d[0:P, 0:CHUNK])
            nc.sync.dma_start(out=self.wk_sb[:, 1:KT_H, :],
                              in_=wk_r[:, 1:KT_H, :])
            for kt in range(1, KT_H):
                nc.sync.dma_start(
                    out=self.xT_sb[:, kt, 0:CHUNK],
                    in_=self.xT_d[kt * P:(kt + 1) * P, 0:CHUNK])
            nc.sync.dma_start(out=self.wq_sb[:], in_=wq_r[:])
            nc.sync.dma_start(out=self.wv_sb[:], in_=wv_r[:])
            for cc in range(1, NCH):
                nc.sync.dma_start(
                    out=self.xT_sb[:, :, cc * CHUNK:(cc + 1) * CHUNK],
                    in_=xT_r[:, :, cc * CHUNK:(cc + 1) * CHUNK])
            nc.sync.dma_start(out=self.wo_sb[:],
                              in_=self.wo_d.rearrange("(t p) c -> p t c", p=P))
            nc.sync.dma_start(out=self.wo2_sb[:],
                              in_=self.wo_d[P + 64:HD, :])

            # ---------- pools ----------
            attn_stack = contextlib.ExitStack()
            self.a_ps = attn_stack.enter_context(
                tc.tile_pool(name="attn_psum", bufs=1, space="PSUM"))
            self.ptp = attn_stack.enter_context(
                tc.tile_pool(name="pt_pool", bufs=12))
            self.npool = attn_stack.enter_context(
                tc.tile_pool(name="norm_pool", bufs=2))
            self.opool = attn_stack.enter_context(
                tc.tile_pool(name="o_pool", bufs=1))
            self.psA = tc.alloc_tile_pool(name="proj_psum", bufs=1,
                                          space="PSUM")

            # ---------- phase A: projections + attention chunk 0 of
            # pair 0, pipelined into the DMA window ----------
            state00 = {"ctx": None}
            cb00 = self.make_norm_cb(0, 0)
            for cc in range(NCH):
                self.qk_cc_proj(self.wk_sb, self.bk_sb, self.kT_sb, 0, cc,
                                "ps_k")
                if cc < 2:
                    # pair-1 K for this column chunk rides in phase A's
                    # DMA-stall windows (needs only wk + this x chunk)
                    self.qk_cc_proj(self.wk_sb, self.bk_sb, self.kT_sb, 1,
                                    cc, "ps_q")
                self.qk_cc_proj(self.wq_sb, self.bq_sb, self.qT_sb, 0, cc,
                                "ps_q" if cc >= 2 else "ps_k")
                for i, m in enumerate(range(4 * cc, 4 * cc + 4)):
                    self.v_one(m, "ps_k" if i % 2 == 0 else "ps_q")
                for kt in range(4 * cc, 4 * cc + 4):
                    self.attn_step(0, 0, kt, state00,
                                   after=(cb00 if kt == KT_S - 1 else None))

            # ---------- phase B: pair-0 chunks 1-3, pair-1 projections
            # as in-loop filler (Q cc2/cc3 reserved for phase C chunk 0,
            # which otherwise has no filler) ----------
            self.queue_kq_pair1([("k", 2), ("k", 3),
                                 ("q", 0), ("q", 1)])
            for c in range(1, NCH):
                self.run_chunk(0, c)
            wv_stack.close()

            # ---------- phase C: pair-1 chunks; each chunk's norm
            # callback queues its output projection as the next chunk's
            # filler ----------
            self.queue_kq_pair1([("q", 2), ("q", 3)])
            for c in range(NCH):
                if c < NCH - 1:
                    self.run_chunk(1, c, oproj_q=c)
                else:
                    self.run_chunk(1, c, split=True)

            # drain the pipeline: last ctx matmuls with filler between,
            # then the deferred norm + final output projection
            for _ in range(10):
                self.emit_pend_ctx()
                self.filler_step(2600.0)
            self.drain_filler()
            self.psA.release()
            w_pool.release()
            attn_stack.close()


def _emit(tc, nc, dram, masked, with_bias):
    _Emitter(tc, nc, dram, masked, with_bias).emit()


def build_program(masked=False, with_bias=False):
    key = (masked, with_bias)
    if key in _PROGRAM_CACHE:
        return _PROGRAM_CACHE[key]
    nc = bacc.Bacc("TRN2", target_bir_lowering=False, debug=False,
                   enable_asserts=False)
    xT = nc.dram_tensor("xT", [H, S], BF16, kind="ExternalInput").ap()
    wq = nc.dram_tensor("wq", [H, HD], BF16, kind="ExternalInput").ap()
    wk = nc.dram_tensor("wk", [H, HD], BF16, kind="ExternalInput").ap()
    wv = nc.dram_tensor("wv", [H, HD], BF16, kind="ExternalInput").ap()
    wo = nc.dram_tensor("wo", [HD, H], F32R, kind="ExternalInput").ap()
    bq = nc.dram_tensor("bq", [1, HD], BF16, kind="ExternalInput").ap()
    bk = nc.dram_tensor("bk", [1, HD], BF16, kind="ExternalInput").ap()
    bv = nc.dram_tensor("bv", [1, HD], BF16, kind="ExternalInput").ap()
    am = nc.dram_tensor("am", [P, KT_S], F32, kind="ExternalInput").ap()
    o = nc.dram_tensor("o_part", [S, H], BF16, kind="ExternalOutput").ap()
    with tile.TileContext(nc) as tc:
        _emit(tc, nc, (xT, wq, wk, wv, wo, bq, bk, bv, am, o), masked, with_bias)
    nc.compile()
    _PROGRAM_CACHE[key] = nc
    return nc


def _round_fp32r(a):
    """Round fp32 to the PE's fp32r format (11 mantissa bits, RNE)."""
    u = np.ascontiguousarray(a, np.float32).view(np.uint32)
    r = (u + np.uint32(0x7FF) + ((u >> np.uint32(12)) & np.uint32(1))) \
        & np.uint32(0xFFFFF000)
    return r.view(np.float32)


def _bf16(a):
    import ml_dtypes
    return np.ascontiguousarray(np.asarray(a, np.float32)).astype(
        ml_dtypes.bfloat16)


def make_in_maps(hidden_states, attention_mask, Wq, bq, Wk, bk, Wv, bv, Wo, bo):
    """Per-core input dicts. Core c: batch c//4, head-group c%4.

    Wq/bq are pre-scaled by 1/8 (= 1/sqrt(HEAD_DIM), exact in fp32) so the
    kernel's raw scores are already scaled. x and Wk/Wq/Wv ship as bf16;
    Wo ships as fp32r (pre-rounded on host).
    """
    hidden_states = np.asarray(hidden_states, np.float32)
    attention_mask = np.asarray(attention_mask, np.float32)
    xTs = [_bf16(hidden_states[b].T) for b in range(B)]
    ams = []
    for b in range(B):
        amask = ((1.0 - attention_mask[b]) * -10000.0).astype(np.float32)
        ams.append(np.ascontiguousarray(amask.reshape(KT_S, P).T))
    in_maps = []
    for c in range(N_CORES):
        b, g = divmod(c, GROUPS)
        hs = slice(g * HD, (g + 1) * HD)
        in_maps.append({
            "xT": xTs[b],
            "wq": _bf16(np.asarray(Wq, np.float32)[hs, :].T * np.float32(0.125)),
            "wk": _bf16(np.asarray(Wk, np.float32)[hs, :].T),
            "wv": _bf16(np.asarray(Wv, np.float32)[hs, :].T),
            "wo": _round_fp32r(np.asarray(Wo, np.float32)[:, hs].T),
            "bq": _bf16(np.asarray(bq, np.float32)[hs].reshape(1, HD) * np.float32(0.125)),
            "bk": _bf16(np.asarray(bk, np.float32)[hs].reshape(1, HD)),
            "bv": _bf16(np.asarray(bv, np.float32)[hs].reshape(1, HD)),
            "am": ams[b],
        })
    return in_maps


def kernel(hidden_states, attention_mask, Wq, bq, Wk, bk, Wv, bv, Wo, bo):
    masked = not bool(np.all(np.asarray(attention_mask) == 1.0))
    with_bias = not (np.all(np.asarray(bq) == 0) and np.all(np.asarray(bk) == 0)
                     and np.all(np.asarray(bv) == 0))
    nc = build_program(masked, with_bias)
    in_maps = make_in_maps(hidden_states, attention_mask,
                           Wq, bq, Wk, bk, Wv, bv, Wo, bo)
    res = run_bass_kernel_spmd(nc, in_maps, core_ids=list(range(N_CORES)))
    out = np.zeros((B, S, H), np.float32)
    for c in range(N_CORES):
        b = c // GROUPS
        out[b] += np.asarray(res.results[c]["o_part"], np.float32)
    out += np.asarray(bo, np.float32)
    return out
